# revision 43
# baseline (speedup 1.0000x reference)
"""Block-diagonal grouped GEMM (GroupLinear) on 8 TRN2 NeuronCores, int8 I/O.

Problem: x [8, 2048, 4096] f32, W [4096, 4096] f32 where only the 64
diagonal 64x64 blocks of W are used:
    y[b,s, g*64+o] = sum_i x[b,s, g*64+i] * W[g*64+o, g*64+i]

HBM-bandwidth bound. The rel-err budget (2e-2) admits int8 transport:
  - Host quantizes x per (token, group): sx = max|x_group|/127,
    xq = round(x/sx) int8  (~0.6% rel err).
  - Weight rows are prescaled by c[g,o] = 127/(R*||Wrow||*rms(xq)) so the
    f32 PSUM accumulator lands in int8 range (R=4.5 sigma; the convert
    rounds-to-nearest and saturates -- measured on HW).  W' stays f16.
  - Device: load xq (mostly int8), upcast int8->f16 on Vector (2x mode),
    f16 matmul -> PSUM f32, convert PSUM f32 -> int8 directly on
    Vector/Scalar, store yq int8.  Host dequant: y = yq * sx / c.
    ~1.2% rel err total.

The PSUM->int8 out-convert stream (65.5k per-partition elems at 1
elem/cycle) plus the int8->f16 in-convert (2x) saturate Vector+Scalar;
every other resource has slack.  So F16PAIRS strip-pairs of x ride the
(otherwise idle) load ring as f16 -- the values are the same integer
xq, exactly representable -- skipping their in-convert and buying
Vector time at zero accuracy cost.

Pipeline (strip = 128 channels x 2048 tokens; 4 matmuls of N=512):
  - Loads are emitted first (HWDGE queues are FIFO per engine): early
    pairs ride the Scalar ring (issue time hides in the fill phase),
    the rest the Sync ring.  Bulk y stores ride the GPSIMD SWDGE ring;
    the last pairs store per-strip on Sync/Scalar so the SWDGE queue
    drains before the final barrier.  Bulk weights ride SWDGE too
    (idle until stores begin); a 32KB leader on Scalar unblocks the
    first matmuls.
  - PSUM is one 8-bank tile; strip s uses half s%2, so out-converts of
    strip s-1 overlap matmuls of strip s on disjoint banks.
  - Out-converts per strip: Vector [0:OUT_V], Scalar [OUT_V:2048].
    Scalar streams one ACTIVATE per strip back-to-back (~1.55us); it is
    the critical resource, so OUT_V balances Vector (which also runs
    the in-converts, strip-granular with 2-strip lookahead emitted
    ahead of its out-slices -- no head-of-line blocking).
  - Measured notes: matmuls pipeline at 215ns when the PE activity
    monitor holds full clock; dep-free filler ops get hoisted by the
    tile scheduler (useless); DVE ops pay a pipe-drain shadow, so big
    Vector out-slices are costlier than Scalar ones.
"""

import numpy as np

import concourse.bacc as bacc
import concourse.mybir as mybir
from concourse.tile import TileContext
from concourse.bass_utils import run_bass_kernel_spmd

B, S, C = 8, 2048, 4096
G, GS = 64, 64            # groups, group size (=in_scale=out_scale)
NSTRIP = C // 128         # 32 strips of 128 channels (2 groups each)
NPAIR = NSTRIP // 2
TOK = 512                 # matmul moving free dim (PSUM bank = 512 f32)
F16 = mybir.dt.float16
FP32 = mybir.dt.float32
I8 = mybir.dt.int8

R_SIGMA = 4.5             # accumulator headroom in sigmas before int8 clip
OUT_V = 384               # per strip: Vector converts PSUM cols [0:OUT_V]
NFILL = 0                 # dep-free ldweights fillers per strip (PE warmth)
F16PAIRS = (3, 6, 9, 12)  # strip-pairs whose x rides f16 (no in-convert)


def _pair_dtypes():
    return ["f16" if p in F16PAIRS else "i8" for p in range(NPAIR)]


def _build_program():
    nc = bacc.Bacc()
    kinds = _pair_dtypes()
    n8 = kinds.count("i8") * 2
    n16 = kinds.count("f16") * 2
    xp8 = nc.declare_dram_parameter("xp8", [128, n8 * S], I8, isOutput=False)
    xp16 = nc.declare_dram_parameter("xp16", [128, n16 * S], F16, isOutput=False)
    wb = nc.declare_dram_parameter("wb", [128, NSTRIP * 128], F16, isOutput=False)
    yp = nc.declare_dram_parameter("yp", [128, NSTRIP * S], I8, isOutput=True)

    with TileContext(nc) as tc:
        with (
            tc.tile_pool(name="wpool", bufs=1) as wpool,
            tc.tile_pool(name="xpool", bufs=8) as xpool,
            tc.tile_pool(name="x16pool", bufs=4) as x16pool,
            tc.tile_pool(name="fpool", bufs=4) as fpool,
            tc.tile_pool(name="opool", bufs=6) as opool,
            tc.tile_pool(name="ppool", bufs=1, space="PSUM") as ppool,
        ):
            # Weights staggered so strip 0 unblocks the first matmuls
            # fast; bulk follows.  All on the Scalar ring, issued first.
            w_sb = wpool.tile([128, NSTRIP * 128], F16)
            nc.scalar.dma_start(out=w_sb[:, :128], in_=wb[:, :128])
            # Bulk weights ride the SWDGE ring, idle until stores begin.
            nc.gpsimd.dma_start(out=w_sb[:, 128:], in_=wb[:, 128:])

            # All loads next, alternating Sync/Scalar rings (each ring
            # carries half the bytes; issue time hides in the fill phase).
            pair_tile = []
            off8 = off16 = 0
            for p in range(NPAIR):
                ring = nc.scalar if p in (1, 3) else nc.sync
                if kinds[p] == "i8":
                    x_t = xpool.tile([128, 2 * S], I8, name="x_t")
                    if p == 0:
                        ring.dma_start(out=x_t[:, :S], in_=xp8[:, :S])
                        ring.dma_start(out=x_t[:, S:], in_=xp8[:, S : 2 * S])
                    else:
                        ring.dma_start(
                            out=x_t[:], in_=xp8[:, off8 : off8 + 2 * S]
                        )
                    off8 += 2 * S
                else:
                    x_t = x16pool.tile([128, 2 * S], F16, name="fx_t")
                    ring.dma_start(
                        out=x_t[:], in_=xp16[:, off16 : off16 + 2 * S]
                    )
                    off16 += 2 * S
                pair_tile.append(x_t)

            # One 8-bank PSUM tile; strip s uses half s%2 (banks 0-3/4-7).
            P = ppool.tile([128, 2 * S], FP32)

            def rhs_for(s):
                p, j = divmod(s, 2)
                if kinds[p] == "f16":
                    return pair_tile[p][:, j * S : (j + 1) * S]
                f_t = fpool.tile([128, S], F16, name="f_t")
                nc.vector.tensor_copy(
                    out=f_t[:], in_=pair_tile[p][:, j * S : (j + 1) * S]
                )
                return f_t

            f_ts = [rhs_for(0), rhs_for(1)]

            o_tiles = {}
            for s in range(NSTRIP):
                p, j = divmod(s, 2)
                if j == 0:
                    o_tiles[p] = opool.tile([128, 2 * S], I8, name="o_t")
                f_t = f_ts[s]
                # Vector lookahead: in-convert strip s+2 ahead of this
                # strip's out-slice in Vector program order.
                if s + 2 < NSTRIP:
                    f_ts.append(rhs_for(s + 2))
                H = (s % 2) * S
                for q in range(S // TOK):
                    nc.tensor.matmul(
                        out=P[:, H + q * TOK : H + (q + 1) * TOK],
                        lhsT=w_sb[:, s * 128 : (s + 1) * 128],
                        rhs=f_t[:, q * TOK : (q + 1) * TOK],
                        start=True,
                        stop=True,
                    )
                o_t = o_tiles[p]
                base = j * S
                nc.vector.tensor_copy(
                    out=o_t[:, base : base + OUT_V], in_=P[:, H : H + OUT_V]
                )
                nc.scalar.copy(
                    out=o_t[:, base + OUT_V : base + S],
                    in_=P[:, H + OUT_V : H + S],
                )
                c0 = 2 * p
                if p >= NPAIR - 3:
                    # Drain tail: store per strip, off the SWDGE ring so
                    # its queue drains before the final barrier.
                    eng = nc.sync if s % 2 == 0 else nc.scalar
                    eng.dma_start(
                        out=yp[:, (c0 + j) * S : (c0 + j + 1) * S],
                        in_=o_t[:, j * S : (j + 1) * S],
                    )
                elif j == 1:
                    nc.gpsimd.dma_start(
                        out=yp[:, c0 * S : (c0 + 2) * S], in_=o_t[:]
                    )
    nc.finalize()
    return nc


def _prep_in_maps(x, W):
    # Diagonal blocks: Wdiag[g][o, i] = W[g*64+o, g*64+i]
    Wr = W.reshape(G, GS, G, GS)
    g = np.arange(G)
    Wd = Wr[g, :, g, :]                                   # [g, o, i]
    rownorm = np.linalg.norm(Wd, axis=2)                  # [g, o]
    rownorm = np.maximum(rownorm, 1e-12)
    kinds = _pair_dtypes()

    maps = []
    dequants = []
    for b in range(B):
        xg = x[b].reshape(S, G, GS)
        sx = np.abs(xg).max(axis=2) / 127.0               # [S, G]
        sx = np.maximum(sx, 1e-30)
        xq = np.rint(xg / sx[:, :, None]).astype(np.int8)  # [S, G, GS]
        rms = float(np.sqrt(np.mean(np.square(xq.astype(np.float32)))))
        c = 127.0 / (R_SIGMA * rownorm * rms)             # [g, o]
        WdT = (Wd * c[:, :, None]).transpose(0, 2, 1).astype(np.float16)  # [g,i,o]
        wb = np.zeros((128, NSTRIP, 128), dtype=np.float16)
        for cs in range(NSTRIP):
            wb[0:64, cs, 0:64] = WdT[2 * cs]
            wb[64:128, cs, 64:128] = WdT[2 * cs + 1]
        wb = np.ascontiguousarray(wb.reshape(128, NSTRIP * 128))
        # strip-major packing: strip cs occupies [p, cs*S + t] with
        # xq[t, channel cs*128+p]; int8 and f16 strips go to separate
        # tensors in pair order.
        xq_s = xq.reshape(S, C).T.reshape(NSTRIP, 128, S)  # [cs, p, t]
        i8_parts, f16_parts = [], []
        for p in range(NPAIR):
            blk = xq_s[2 * p : 2 * p + 2]                  # [2, p, t]
            blk = blk.transpose(1, 0, 2).reshape(128, 2 * S)
            if kinds[p] == "i8":
                i8_parts.append(blk)
            else:
                f16_parts.append(blk.astype(np.float16))
        xp8 = np.ascontiguousarray(np.concatenate(i8_parts, axis=1))
        xp16 = (
            np.ascontiguousarray(np.concatenate(f16_parts, axis=1))
            if f16_parts
            else np.zeros((128, 0), dtype=np.float16)
        )
        maps.append({"xp8": xp8, "xp16": xp16, "wb": wb})
        dequants.append((sx, 1.0 / c))
    return maps, dequants


def run(x, W, trace=False, **kw):
    x = np.asarray(x, dtype=np.float32)
    W = np.asarray(W, dtype=np.float32)
    nc = _build_program()
    in_maps, dequants = _prep_in_maps(x, W)
    res = run_bass_kernel_spmd(nc, in_maps, list(range(B)), trace=trace, **kw)
    y = np.empty((B, S, C), dtype=np.float32)
    for b in range(B):
        yp = res.results[b]["yp"]
        sx, inv_c = dequants[b]
        # y[t, cs*128 + p] = yp[p, cs*S + t] * sx[t, g] * inv_c[g, o]
        # (sx = max/127 already folds the 127)
        z = (
            yp.reshape(128, NSTRIP, S)
            .transpose(2, 1, 0)                           # [t, cs, p]
            .reshape(S, G, GS)
            .astype(np.float32)
        )
        yb = z * inv_c[None, :, :] * sx[:, :, None]
        y[b] = yb.reshape(S, C)
    return y, res


def kernel(x, W):
    y, _ = run(x, W, trace=False)
    return y
